# revision 25
# baseline (speedup 1.0000x reference)
"""DPLR-SSM block kernel for Trainium2 (8 NeuronCores, batch-data-parallel).

Computes, for the full inputs:
    xB = einsum("bth,hk->btk", x, B)
    h_{t+1} = tanh(d * h_t + (h_t @ R.T) @ L.T + xB[:, t])   (scan over t)
    out[:, t] = h_{t+1}

Sharding: batch 128 -> 16 per core (8 cores), params replicated.

Per-core device layout ("layout A"): state h lives in an SBUF tile
[128, 128] indexed [p, hb*16 + b] with h-index = hb*128 + p (b = local
batch, hb = h-block).  Per scan step:
  - y_rep [32,16] PSUM  = 8 PSUM-accumulated matmuls with column-replicated
    R weights  (y = R @ h, replicated over 8 partition groups)
  - bd [32,(8,16)] SBUF = broadcast(y_rep) * blockdiag_mask  (one DVE op)
  - lr [128,128] PSUM   = W2.T @ bd  (one matmul, constant [32,128] L weights)
                          += I.T @ u_t  (identity matmul, u from fused GEMM)
                          += I.T @ (d*h) (identity matmul; d*h on DVE)
  - h' = tanh(lr) on ScalarE, PSUM -> SBUF
  - PE-transpose h' -> PSUM -> copy -> SBUF -> DMA to out[b, t, :]
The xB GEMM is emitted interleaved with the scan so its matmuls fill the
PE idle slots of the latency-bound recurrence; u chunks (32 timesteps)
are double-buffered in SBUF and never round-trip through DRAM.

The GEMM runs in bf16 (x, B, and the W2/bd low-rank operands are rounded
host-side; PSUM accumulation stays fp32): trn2's PE streams fp32 moving
data at 1/4 rate (4 cycles/column), so bf16 cuts the dominant GEMM cost
4x. The recurrence state h and all elementwise math stay fp32; the
contractive recurrence keeps the bf16 input error saturated at ~1.3e-2
absmax-relative (verified at T=32/96/256 -- no growth with T).
"""

import sys

sys.path.insert(0, "/opt/trn_rl_repo")

import numpy as np

import concourse.bass as bass
import concourse.mybir as mybir
import concourse.tile as tile
from concourse import bacc
from concourse.bass_utils import run_bass_kernel_spmd

H = 1024
RANK = 4
BATCH = 128
T = 256
NCORES = 8
BL = BATCH // NCORES  # 16 local batches
HB = H // 128  # 8 h-blocks
CH = 32  # timesteps per GEMM chunk
WOUT = 4  # timesteps per output DMA window
FP32 = mybir.dt.float32
BF16 = mybir.dt.bfloat16


def build_program(
    n_steps=T,
    fused=True,
    strip=False,
    loops=1,
    timing_reps=0,
    no_gemm=False,
    no_out=False,
    y_one=False,
    gemm_split=2,
):
    """Build the single-core SPMD bass program."""
    ch = min(CH, n_steps)
    wout = min(WOUT, n_steps)
    assert n_steps % ch == 0
    assert n_steps % wout == 0
    nchunk = n_steps // ch
    nc = bacc.Bacc()

    # ---- DRAM I/O (per-core shard + host-preprocessed constants) ----
    if timing_reps:
        xT_d = nc.dram_tensor("xT", [HB, 128, ch * BL], BF16, kind="ExternalInput")
    else:
        xT_d = nc.dram_tensor(
            "xT", [HB, 128, n_steps * BL], BF16, kind="ExternalInput"
        )
    Bw_d = nc.dram_tensor("Bw", [128, HB, H], BF16, kind="ExternalInput")
    W1a_d = nc.dram_tensor("W1a", [128, 32], FP32, kind="ExternalInput")
    P32_d = nc.dram_tensor("P32", [32, 32], FP32, kind="ExternalInput")
    W2_d = nc.dram_tensor("W2", [32, 128], BF16, kind="ExternalInput")
    mask_d = nc.dram_tensor("mask", [32, HB, BL], FP32, kind="ExternalInput")
    dbc_d = nc.dram_tensor("dbc", [128, 128], FP32, kind="ExternalInput")
    id_d = nc.dram_tensor("ident", [128, 128], FP32, kind="ExternalInput")
    h0_d = nc.dram_tensor("h0A", [128, 128], FP32, kind="ExternalInput")
    okind = "Internal" if timing_reps else "ExternalOutput"
    out_d = nc.dram_tensor("out", [BL, n_steps, H], FP32, kind=okind)
    if timing_reps:
        tok_d = nc.dram_tensor("token", [1, 4], FP32, kind="ExternalOutput")
    scr_d = nc.dram_tensor("oscr", [n_steps // wout, 128, wout, 128], FP32)

    import contextlib

    with tile.TileContext(nc) as tc:
        loop_cm = (
            tc.For_i(
                0,
                loops,
                1,
                hint_engines=(
                    mybir.EngineType.PE,
                    mybir.EngineType.DVE,
                    mybir.EngineType.Activation,
                    mybir.EngineType.Pool,
                    mybir.EngineType.SP,
                ),
            )
            if loops > 1
            else contextlib.nullcontext()
        )
        with (
            tc.tile_pool(name="consts", bufs=1) as consts,
            tc.tile_pool(name="xt", bufs=24) as xtp,
            tc.tile_pool(name="uc", bufs=3) as ucp,
            tc.tile_pool(name="h", bufs=8) as hp,
            tc.tile_pool(name="dh", bufs=2) as dhp,
            tc.tile_pool(name="bd", bufs=2) as bdp,
            tc.tile_pool(name="sps", bufs=2, space="PSUM") as sp,
            tc.tile_pool(name="ho", bufs=2) as hop,
            tc.tile_pool(name="gps", bufs=2, space="PSUM") as gps,
            tc.tile_pool(name="yps", bufs=1, space="PSUM") as yps,
            tc.tile_pool(name="lps", bufs=1, space="PSUM") as lps,
            tc.tile_pool(name="tps", bufs=1, space="PSUM") as tps,
            tc.tile_pool(name="dps", bufs=1, space="PSUM") as dps,
        ):
            # ---- load constants ----
            B_sb = consts.tile([128, HB, H], BF16)
            nc.sync.dma_start(B_sb[:], Bw_d[:])
            W1a_sb = consts.tile([128, 32], FP32)
            nc.sync.dma_start(W1a_sb[:], W1a_d[:])
            P32_sb = consts.tile([32, 32], FP32)
            nc.sync.dma_start(P32_sb[:], P32_d[:])
            W2_sb = consts.tile([32, 128], BF16)
            nc.sync.dma_start(W2_sb[:], W2_d[:])
            mask_sb = consts.tile([32, HB, BL], FP32)
            nc.sync.dma_start(mask_sb[:], mask_d[:])
            dbc_sb = consts.tile([128, 128], FP32)
            nc.sync.dma_start(dbc_sb[:], dbc_d[:])
            I_sb = consts.tile([128, 128], FP32)
            nc.sync.dma_start(I_sb[:], id_d[:])
            h_prev = hp.tile([128, 128], FP32, tag="h")
            nc.sync.dma_start(h_prev[:], h0_d[:])

            # PE "wait absorber" touches: matmul can carry only ONE sync
            # wait on trn2, so teach PE's vector clock about every const
            # DMA queue up front (one dummy matmul per const).
            dummy_ps = dps.tile([128, 128], FP32, tag="dps")
            for cap in (B_sb[:, 0, 0:128], W1a_sb[:], P32_sb[:], W2_sb[:],
                        mask_sb[:], dbc_sb[:], I_sb[:], h_prev[:]):
                fs = cap.free_size()
                nc.tensor.matmul(dummy_ps[:fs, :fs], cap, cap, start=True, stop=True)
            # DVE/ACT const-touch absorbers (each carries one DMA-queue wait)
            sc1 = consts.tile([128, 1], FP32)
            sc2 = consts.tile([128, 1], FP32)
            sc3 = consts.tile([128, 1], FP32)
            nc.vector.tensor_copy(sc1[:1, :], dbc_sb[:1, :1])
            nc.vector.tensor_copy(sc2[:1, :], mask_sb[:1, :1, 0])
            nc.scalar.copy(sc3[:1, :], dbc_sb[:1, :1])
            zb = consts.tile([128, 1], FP32)
            nc.vector.memset(zb[:], 0.0)
            sc4 = consts.tile([128, 1], FP32)
            nc.scalar.copy(sc4[:1, :], h_prev[:1, :1])

            # ---- GEMM chunk emission (generator yields after each item) ----
            loop_ctx = loop_cm.__enter__()
            for _rep in range(max(1, timing_reps)):
                pdum = dps.tile([128, 128], FP32, tag="dps")
                u_tiles = [None] * nchunk

                def emit_chunk(c, prologue=False):
                    xts = []
                    for hbk in range(HB):
                        xt = xtp.tile([128, ch * BL], BF16, tag="xt")
                        xsl = (
                            xT_d[hbk, :, :]
                            if timing_reps
                            else xT_d[hbk, :, c * ch * BL : (c + 1) * ch * BL]
                        )
                        nc.sync.dma_start(xt[:], xsl)
                        xts.append(xt)
                        yield
                    u_tile = ucp.tile([128, HB, ch, BL], FP32, tag="uc")
                    u_tiles[c] = u_tile
                    hc = ch // gemm_split
                    for hbp in range(HB):
                        ps = gps.tile([128, ch, BL], FP32, tag="gps")
                        for hbk in range(HB):
                            for hf in range(gemm_split):
                                nc.tensor.matmul(
                                    ps[:, hf * hc : (hf + 1) * hc, :],
                                    B_sb[:, hbk, hbp * 128 : (hbp + 1) * 128],
                                    xts[hbk][:, hf * hc * BL : (hf + 1) * hc * BL],
                                    start=(hbk == 0 and hf == 0),
                                    stop=(hbk == HB - 1 and hf == gemm_split - 1),
                                )
                                yield
                        # copy psum -> u_tile[:, hbp] in halves on DVE (cheap
                        # engine in this env; avoids ACT<->DVE flip-flops)
                        q = ch // 2
                        for j in range(2):
                            dst = u_tile[:, hbp, j * q : (j + 1) * q, :]
                            piece = ps[:, j * q : (j + 1) * q, :]
                            nc.vector.tensor_copy(dst, piece)
                        if prologue:
                            # PE absorber: observe the copy's ACT tick so later
                            # GEMM matmuls' bank-WAR waits are pre-satisfied
                            nc.tensor.matmul(
                                dummy_ps[:32, :BL],
                                W1a_sb[:],
                                u_tile[:, hbp, 0, :],
                                start=True,
                                stop=True,
                            )
                        yield

                gemm_work = []  # list of generators, consumed round-robin

                def pump_gemm(n):
                    done = 0
                    while done < n and gemm_work:
                        try:
                            next(gemm_work[0])
                            done += 1
                        except StopIteration:
                            gemm_work.pop(0)

                # prologue: only chunk 0 before the scan; chunk 1 streams
                # during steps 0..ch-1 (needed first at step ch)
                gemm_work.append(emit_chunk(0, prologue=True))
                pump_gemm(10**9)
                if no_gemm:
                    for cc_ in range(1, nchunk):
                        u_tiles[cc_] = u_tiles[0]
                    next_chunk = nchunk
                elif nchunk > 1:
                    gemm_work.append(emit_chunk(1, prologue=True))
                    next_chunk = 2
                else:
                    next_chunk = 2

                # ---- the scan ----
                TANH = mybir.ActivationFunctionType.Tanh
                hobufs = [None, None]
                h_hist = {}

                def emit_out(t):
                    wl = t % wout
                    if wl == 0:
                        hobuf = hop.tile([128, wout, 128], FP32, tag="ho")
                        hobufs[(t // wout) % 2] = hobuf
                        # absorber: carry the stage-DMA's queue wait on a
                        # lone ACT op (every instr gets at most one sync wait)
                        nc.scalar.copy(hobuf[:1, 0, :1], dbc_sb[:1, :1])
                    hobuf = hobufs[(t // wout) % 2]
                    ht = tps.tile([128, 128], FP32, tag="tps")
                    nc.tensor.transpose(ht[:], h_hist.pop(t)[:], I_sb[:])
                    nc.scalar.copy(hobuf[:, wl, :], ht[:])
                    if wl == wout - 1:
                        w0, w = t - wl, t // wout
                        nc.sync.dma_start(scr_d[w], hobuf[:])
                        dst4 = out_d[:, w0 : w0 + wout, :].rearrange(
                            "b tl (hb p) -> hb b tl p", hb=HB
                        )
                        scr4 = scr_d[w].rearrange("(hb b) tl p -> hb b tl p", hb=HB)
                        for hb in range(HB):
                            nc.sync.dma_start(dst4[hb], scr4[hb])
                for t in range(n_steps):
                    c, tl = t // ch, t % ch
                    if tl == 0 and next_chunk < nchunk:
                        gemm_work.append(emit_chunk(next_chunk))
                        next_chunk += 1
                        if not fused:
                            pump_gemm(10**9)

                    u_tile = u_tiles[c]
                    u_ap = u_tile[:, :, tl, :]  # [128, HB, BL] strided

                    dh = dhp.tile([128, 128], FP32, tag="dh")
                    nc.vector.tensor_mul(dh[:], h_prev[:], dbc_sb[:])

                    # ya[(hbr,r),(hbk,b)] = R-block-hbr . h-block-hbk: ONE
                    # indep matmul (no PSUM-accum chain -- chained accumulating
                    # matmuls cost ~5x in this env)
                    y_ps_full = yps.tile([128, 4, HB, BL], FP32, tag="yps")
                    ya = y_ps_full[:32, 0, :, :]
                    y_rep = y_ps_full[:32, 2, 0, :]
                    nc.tensor.matmul(ya, W1a_sb[:], h_prev[:], start=True, stop=True)
                    # mask off-diagonal block pairs, then col-tree-sum the HB
                    # groups: s8[(hbr,r),b] = partial_hbr[r,b]  (4 DVE ops;
                    # contiguous-stride adds beat a strided tensor_reduce here)
                    yt = bdp.tile([32, HB, BL], FP32, tag="m1")
                    tr = bdp.tile([32, 7, BL], FP32, tag="tr")
                    nc.vector.tensor_mul(yt[:], ya, mask_sb[:])
                    nc.vector.tensor_add(tr[:, 0:4, :], yt[:, 0:4, :], yt[:, 4:8, :])
                    nc.vector.tensor_add(tr[:, 4:6, :], tr[:, 0:2, :], tr[:, 2:4, :])
                    nc.vector.tensor_add(tr[:, 6:7, :], tr[:, 4:5, :], tr[:, 5:6, :])
                    # y_rep[(hbr',r),b] = sum_hbr s8[(hbr,r),b] = y[r,b] (tiny mm)
                    nc.tensor.matmul(
                        y_rep, P32_sb[:], tr[:, 6, :], start=True, stop=True
                    )

                    # bd = broadcast(y_rep) * mask  (block-diagonal [32, HB, BL])
                    bd = bdp.tile([32, HB, BL], BF16, tag="bd")
                    yap = y_rep
                    y_b = bass.AP(
                        tensor=yap.tensor,
                        offset=yap.offset,
                        ap=[yap.ap[0], [0, HB], yap.ap[1]],
                    )
                    bd_i = nc.vector.tensor_mul(bd[:], y_b, mask_sb[:])
                    s1 = dhp.tile([128, 128], FP32, tag="s1")
                    s1_i = nc.vector.tensor_add(s1[:], dh[:], u_ap)
                    # keep bd ahead of s1 on DVE: bd feeds the lr matmul on
                    # the critical path; s1 has slack until s2
                    tile.add_dep_helper(bd_i.ins, s1_i.ins, sync=False, reason="bd first")

                    lr = lps.tile([128, 128], FP32, tag="lps")
                    nc.tensor.matmul(lr[:], W2_sb[:], bd[:], start=True, stop=True)

                    # output path for the PREVIOUS step (keeps the PE transpose
                    # off this step's tanh -> y-matmul critical path)
                    if t > 0 and not no_out:
                        emit_out(t - 1)

                    # s = (dh + u) + lr on DVE (s1 runs parallel to the matmuls)
                    s_t = sp.tile([128, 128], FP32, tag="s")
                    nc.vector.tensor_add(s_t[:], s1[:], lr[:])

                    h_new = hp.tile([128, 128], FP32, tag="h")
                    nc.scalar.activation(h_new[:], s_t[:], TANH, bias=zb[:])

                    if not no_out:
                        h_hist[t] = h_new
                    h_prev = h_new
                    if fused:
                        pump_gemm(6)
                if not no_out:
                    emit_out(n_steps - 1)
                pump_gemm(10**9)
            if timing_reps:
                nc.sync.dma_start(tok_d[:], dbc_sb[:1, :4])
            loop_cm.__exit__(None, None, None)

    if strip:
        _strip_self_waits(nc, dma=(strip is True))
    nc.compile()
    return nc


_ENG_SEM = {
    "EngineType.PE": "PE_",
    "EngineType.DVE": "DVE_",
    "EngineType.Activation": "Activation_",
}


def _strip_self_waits(nc, dma=True):
    """trn2 compute instructions carry at most ONE sync wait.  Engines
    execute and retire their queues strictly in order, so a wait on the
    instruction's own engine semaphore (emitted by Tile for cross-step
    tile reuse) is redundant -- drop those when over the limit.  The
    dma=True mode additionally drops DMA WAW waits (NOT safe for the
    real program's out-DMA windows -- timing-only)."""
    import concourse.mybir as _mb

    over = []
    for b in nc.m.functions[0].blocks:
        for inst in b.instructions:
            si = inst.sync_info
            if not si or not si.on_wait or len(si.on_wait) <= 1:
                continue
            ty = type(inst).__name__
            keep = si.on_wait
            pfx = _ENG_SEM.get(str(getattr(inst, "engine", None)))
            if pfx is not None:
                keep = [w for w in keep if not w.ant_name.startswith(pfx)]
            if dma and ty == "InstDMACopy" and len(keep) > 1:
                # DMA WAW waits on other DMA-queue sems: every recycled DMA
                # target in this kernel is transitively ordered through the
                # kept engine-sem wait (xt: PE readers; hobuf: ACT copies),
                # and DRAM-out windows are disjoint regions.
                eng_waits = [w for w in keep if not w.ant_name.startswith("DMA")]
                if eng_waits:
                    keep = eng_waits
                else:
                    keep = keep[-1:]
            if len(keep) < len(si.on_wait):
                inst.sync_info = _mb.SyncInfo(on_wait=keep, on_update=si.on_update)
            if len(keep) > 1:
                over.append((inst.name, ty, [w.ant_name for w in keep]))
    if over:
        print(f"WARNING: {len(over)} instructions still have >1 wait:")
        for o in over[:10]:
            print("   ", o)


_PROG_CACHE = {}


def build_program_timed(n_steps=T, reps=8, **kw):
    return build_program(n_steps, timing_reps=reps, **kw)


STRIP = "eng"  # drop redundant same-engine sync waits (not DMA WAW ones)


def _get_prog(n_steps=T, fused=True):
    key = (n_steps, fused, STRIP)
    if key not in _PROG_CACHE:
        _PROG_CACHE[key] = build_program(n_steps, fused, strip=STRIP)
    return _PROG_CACHE[key]


def make_core_inputs(x, h0, d, L, R, B, n_steps=T):
    """Host-side preprocessing -> list of per-core input dicts."""
    x = np.asarray(x, np.float32)
    h0 = np.asarray(h0, np.float32)
    d = np.asarray(d, np.float32)
    L = np.asarray(L, np.float32)
    R = np.asarray(R, np.float32)
    B = np.asarray(B, np.float32)

    import ml_dtypes

    bf16 = ml_dtypes.bfloat16
    # constants (replicated across cores)
    Bw = np.ascontiguousarray(B.reshape(HB, 128, H).transpose(1, 0, 2)).astype(bf16)
    # W1a[p, hbr*RANK+r] = R[r, hbr*128+p]  (lhsT for the one-shot ya matmul)
    Rr = R.reshape(RANK, HB, 128)  # [r, hbr, p]
    W1a = np.ascontiguousarray(Rr.transpose(2, 1, 0).reshape(128, 32))
    # P32[(hbr,r),(hbr',r')] = [r==r']: partition-sum of the 8 rank-groups
    P32 = np.tile(np.eye(RANK, dtype=np.float32), (HB, HB))
    # W2[hb2*RANK+r, p] = L[hb2*128+p, r]
    W2 = np.ascontiguousarray(
        L.reshape(HB, 128, RANK).transpose(0, 2, 1).reshape(32, 128)
    ).astype(bf16)
    mask = np.zeros((32, HB, BL), np.float32)
    for hb in range(HB):
        mask[hb * RANK : (hb + 1) * RANK, hb, :] = 1.0
    # dbc[p, hb*BL+b] = d[hb*128+p]
    dbc = np.ascontiguousarray(
        np.repeat(d.reshape(HB, 128).T[:, :, None], BL, axis=2).reshape(128, 128)
    )
    ident = np.eye(128, dtype=np.float32)

    in_maps = []
    for core in range(NCORES):
        sl = slice(core * BL, (core + 1) * BL)
        xs = x[sl, :n_steps]  # [BL, T, H]
        # xT[hbk, k, t*BL+b] = x[b, t, hbk*128+k]
        xT = np.ascontiguousarray(
            xs.reshape(BL, n_steps, HB, 128)
            .transpose(2, 3, 1, 0)
            .reshape(HB, 128, n_steps * BL)
        ).astype(bf16)
        h0s = h0[sl]  # [BL, H]
        h0A = np.ascontiguousarray(
            h0s.reshape(BL, HB, 128).transpose(2, 1, 0).reshape(128, 128)
        )
        in_maps.append(
            {
                "xT": xT,
                "Bw": Bw,
                "W1a": W1a,
                "P32": P32,
                "W2": W2,
                "mask": mask,
                "dbc": dbc,
                "ident": ident,
                "h0A": h0A,
            }
        )
    return in_maps


def gather_output(results, n_steps=T):
    """results: list of per-core dicts with 'out' [BL, T, H] -> [BATCH, T, H]."""
    return np.concatenate([np.asarray(r["out"]) for r in results], axis=0)


def kernel(x, h0, d, L, R, B):
    nc = _get_prog(T)
    in_maps = make_core_inputs(x, h0, d, L, R, B, T)
    res = run_bass_kernel_spmd(nc, in_maps, list(range(NCORES)))
    return gather_output(res.results, T)


if __name__ == "__main__":
    nc = build_program()
    print("built ok:", sum(1 for _ in nc.m.functions[0].body))



# revision 26
# speedup vs baseline: 1.2695x; 1.2695x over previous
"""DPLR-SSM block kernel for Trainium2 (8 NeuronCores, batch-data-parallel).

Computes, for the full inputs:
    xB = einsum("bth,hk->btk", x, B)
    h_{t+1} = tanh(d * h_t + (h_t @ R.T) @ L.T + xB[:, t])   (scan over t)
    out[:, t] = h_{t+1}

Sharding: batch 128 -> 16 per core (8 cores), params replicated.

Per-core device layout ("layout A"): state h lives in an SBUF tile
[128, 128] indexed [p, hb*16 + b] with h-index = hb*128 + p (b = local
batch, hb = h-block).  Per scan step:
  - y_rep [32,16] PSUM  = 8 PSUM-accumulated matmuls with column-replicated
    R weights  (y = R @ h, replicated over 8 partition groups)
  - bd [32,(8,16)] SBUF = broadcast(y_rep) * blockdiag_mask  (one DVE op)
  - lr [128,128] PSUM   = W2.T @ bd  (one matmul, constant [32,128] L weights)
                          += I.T @ u_t  (identity matmul, u from fused GEMM)
                          += I.T @ (d*h) (identity matmul; d*h on DVE)
  - h' = tanh(lr) on ScalarE, PSUM -> SBUF
  - PE-transpose h' -> PSUM -> copy -> SBUF -> DMA to out[b, t, :]
The xB GEMM is emitted interleaved with the scan so its matmuls fill the
PE idle slots of the latency-bound recurrence; u chunks (32 timesteps)
are double-buffered in SBUF and never round-trip through DRAM.

The GEMM runs in bf16 (x, B, and the W2/bd low-rank operands are rounded
host-side; PSUM accumulation stays fp32): trn2's PE streams fp32 moving
data at 1/4 rate (4 cycles/column), so bf16 cuts the dominant GEMM cost
4x. The recurrence state h and all elementwise math stay fp32; the
contractive recurrence keeps the bf16 input error saturated at ~1.3e-2
absmax-relative (verified at T=32/96/256 -- no growth with T).
"""

import sys

sys.path.insert(0, "/opt/trn_rl_repo")

import numpy as np

import concourse.bass as bass
import concourse.mybir as mybir
import concourse.tile as tile
from concourse import bacc
from concourse.bass_utils import run_bass_kernel_spmd

H = 1024
RANK = 4
BATCH = 128
T = 256
NCORES = 8
BL = BATCH // NCORES  # 16 local batches
HB = H // 128  # 8 h-blocks
CH = 32  # timesteps per GEMM chunk
WOUT = 4  # timesteps per output DMA window
FP32 = mybir.dt.float32
BF16 = mybir.dt.bfloat16


def build_program(
    n_steps=T,
    fused=True,
    strip=False,
    loops=1,
    timing_reps=0,
    no_gemm=False,
    no_out=False,
    y_one=False,
    gemm_split=2,
):
    """Build the single-core SPMD bass program."""
    ch = min(CH, n_steps)
    wout = min(WOUT, n_steps)
    assert n_steps % ch == 0
    assert n_steps % wout == 0
    nchunk = n_steps // ch
    nc = bacc.Bacc()

    # ---- DRAM I/O (per-core shard + host-preprocessed constants) ----
    if timing_reps:
        xT_d = nc.dram_tensor("xT", [HB, 128, ch * BL], BF16, kind="ExternalInput")
    else:
        xT_d = nc.dram_tensor(
            "xT", [HB, 128, n_steps * BL], BF16, kind="ExternalInput"
        )
    Bw_d = nc.dram_tensor("Bw", [128, HB, H], BF16, kind="ExternalInput")
    W1a_d = nc.dram_tensor("W1a", [128, 32], FP32, kind="ExternalInput")
    P32_d = nc.dram_tensor("P32", [32, 32], FP32, kind="ExternalInput")
    W2_d = nc.dram_tensor("W2", [32, 128], BF16, kind="ExternalInput")
    mask_d = nc.dram_tensor("mask", [32, HB, BL], FP32, kind="ExternalInput")
    dbc_d = nc.dram_tensor("dbc", [128, 128], FP32, kind="ExternalInput")
    id_d = nc.dram_tensor("ident", [128, 128], FP32, kind="ExternalInput")
    h0_d = nc.dram_tensor("h0A", [128, 128], FP32, kind="ExternalInput")
    okind = "Internal" if timing_reps else "ExternalOutput"
    out_d = nc.dram_tensor("out", [BL, n_steps, H], FP32, kind=okind)
    if timing_reps:
        tok_d = nc.dram_tensor("token", [1, 4], FP32, kind="ExternalOutput")
    scr_d = nc.dram_tensor("oscr", [n_steps // wout, 128, wout, 128], FP32)

    import contextlib

    with tile.TileContext(nc) as tc:
        loop_cm = (
            tc.For_i(
                0,
                loops,
                1,
                hint_engines=(
                    mybir.EngineType.PE,
                    mybir.EngineType.DVE,
                    mybir.EngineType.Activation,
                    mybir.EngineType.Pool,
                    mybir.EngineType.SP,
                ),
            )
            if loops > 1
            else contextlib.nullcontext()
        )
        with (
            tc.tile_pool(name="consts", bufs=1) as consts,
            tc.tile_pool(name="xt", bufs=24) as xtp,
            tc.tile_pool(name="uc", bufs=3) as ucp,
            tc.tile_pool(name="h", bufs=8) as hp,
            tc.tile_pool(name="dh", bufs=2) as dhp,
            tc.tile_pool(name="bd", bufs=2) as bdp,
            tc.tile_pool(name="sps", bufs=2, space="PSUM") as sp,
            tc.tile_pool(name="ho", bufs=2) as hop,
            tc.tile_pool(name="gps", bufs=2, space="PSUM") as gps,
            tc.tile_pool(name="yps", bufs=1, space="PSUM") as yps,
            tc.tile_pool(name="lps", bufs=1, space="PSUM") as lps,
            tc.tile_pool(name="tps", bufs=1, space="PSUM") as tps,
            tc.tile_pool(name="dps", bufs=1, space="PSUM") as dps,
        ):
            # ---- load constants ----
            B_sb = consts.tile([128, HB, H], BF16)
            nc.sync.dma_start(B_sb[:], Bw_d[:])
            W1a_sb = consts.tile([128, 32], FP32)
            nc.sync.dma_start(W1a_sb[:], W1a_d[:])
            P32_sb = consts.tile([32, 32], FP32)
            nc.sync.dma_start(P32_sb[:], P32_d[:])
            W2_sb = consts.tile([32, 128], BF16)
            nc.sync.dma_start(W2_sb[:], W2_d[:])
            mask_sb = consts.tile([32, HB, BL], FP32)
            nc.sync.dma_start(mask_sb[:], mask_d[:])
            dbc_sb = consts.tile([128, 128], FP32)
            nc.sync.dma_start(dbc_sb[:], dbc_d[:])
            I_sb = consts.tile([128, 128], FP32)
            nc.sync.dma_start(I_sb[:], id_d[:])
            h_prev = hp.tile([128, 128], FP32, tag="h")
            nc.sync.dma_start(h_prev[:], h0_d[:])

            # PE "wait absorber" touches: matmul can carry only ONE sync
            # wait on trn2, so teach PE's vector clock about every const
            # DMA queue up front (one dummy matmul per const).
            dummy_ps = dps.tile([128, 128], FP32, tag="dps")
            for cap in (B_sb[:, 0, 0:128], W1a_sb[:], P32_sb[:], W2_sb[:],
                        mask_sb[:], dbc_sb[:], I_sb[:], h_prev[:]):
                fs = cap.free_size()
                nc.tensor.matmul(dummy_ps[:fs, :fs], cap, cap, start=True, stop=True)
            # DVE/ACT const-touch absorbers (each carries one DMA-queue wait)
            sc1 = consts.tile([128, 1], FP32)
            sc2 = consts.tile([128, 1], FP32)
            sc3 = consts.tile([128, 1], FP32)
            nc.vector.tensor_copy(sc1[:1, :], dbc_sb[:1, :1])
            nc.vector.tensor_copy(sc2[:1, :], mask_sb[:1, :1, 0])
            nc.scalar.copy(sc3[:1, :], dbc_sb[:1, :1])
            zb = consts.tile([128, 1], FP32)
            nc.vector.memset(zb[:], 0.0)
            sc4 = consts.tile([128, 1], FP32)
            nc.scalar.copy(sc4[:1, :], h_prev[:1, :1])

            # ---- GEMM chunk emission (generator yields after each item) ----
            loop_ctx = loop_cm.__enter__()
            for _rep in range(max(1, timing_reps)):
                pdum = dps.tile([128, 128], FP32, tag="dps")
                u_tiles = [None] * nchunk

                def emit_chunk(c, prologue=False):
                    xts = []
                    for hbk in range(HB):
                        xt = xtp.tile([128, ch * BL], BF16, tag="xt")
                        xsl = (
                            xT_d[hbk, :, :]
                            if timing_reps
                            else xT_d[hbk, :, c * ch * BL : (c + 1) * ch * BL]
                        )
                        nc.sync.dma_start(xt[:], xsl)
                        xts.append(xt)
                        yield
                    u_tile = ucp.tile([128, HB, ch, BL], FP32, tag="uc")
                    u_tiles[c] = u_tile
                    hc = ch // gemm_split
                    for hbp in range(HB):
                        ps = gps.tile([128, ch, BL], FP32, tag="gps")
                        for hbk in range(HB):
                            for hf in range(gemm_split):
                                nc.tensor.matmul(
                                    ps[:, hf * hc : (hf + 1) * hc, :],
                                    B_sb[:, hbk, hbp * 128 : (hbp + 1) * 128],
                                    xts[hbk][:, hf * hc * BL : (hf + 1) * hc * BL],
                                    start=(hbk == 0 and hf == 0),
                                    stop=(hbk == HB - 1 and hf == gemm_split - 1),
                                )
                                yield
                        # copy psum -> u_tile[:, hbp] in halves on DVE (cheap
                        # engine in this env; avoids ACT<->DVE flip-flops)
                        q = ch // 2
                        for j in range(2):
                            dst = u_tile[:, hbp, j * q : (j + 1) * q, :]
                            piece = ps[:, j * q : (j + 1) * q, :]
                            nc.vector.tensor_copy(dst, piece)
                        if prologue:
                            # PE absorber: observe the copy's ACT tick so later
                            # GEMM matmuls' bank-WAR waits are pre-satisfied
                            nc.tensor.matmul(
                                dummy_ps[:32, :BL],
                                W1a_sb[:],
                                u_tile[:, hbp, 0, :],
                                start=True,
                                stop=True,
                            )
                        yield

                gemm_work = []  # list of generators, consumed round-robin

                def pump_gemm(n):
                    done = 0
                    while done < n and gemm_work:
                        try:
                            next(gemm_work[0])
                            done += 1
                        except StopIteration:
                            gemm_work.pop(0)

                # prologue: only chunk 0 before the scan; chunk 1 streams
                # during steps 0..ch-1 (needed first at step ch)
                gemm_work.append(emit_chunk(0, prologue=True))
                pump_gemm(10**9)
                if no_gemm:
                    for cc_ in range(1, nchunk):
                        u_tiles[cc_] = u_tiles[0]
                    next_chunk = nchunk
                elif nchunk > 1:
                    gemm_work.append(emit_chunk(1, prologue=True))
                    next_chunk = 2
                else:
                    next_chunk = 2

                # ---- the scan ----
                TANH = mybir.ActivationFunctionType.Tanh
                hobufs = [None, None]
                h_hist = {}

                def emit_out(t):
                    wl = t % wout
                    if wl == 0:
                        hobuf = hop.tile([128, wout, 128], FP32, tag="ho")
                        hobufs[(t // wout) % 2] = hobuf
                        # absorber: carry the stage-DMA's queue wait on a
                        # lone ACT op (every instr gets at most one sync wait)
                        nc.scalar.copy(hobuf[:1, 0, :1], dbc_sb[:1, :1])
                    hobuf = hobufs[(t // wout) % 2]
                    ht = tps.tile([128, 128], FP32, tag="tps")
                    nc.tensor.transpose(ht[:], h_hist.pop(t)[:], I_sb[:])
                    nc.scalar.copy(hobuf[:, wl, :], ht[:])
                    if wl == wout - 1:
                        w0, w = t - wl, t // wout
                        nc.sync.dma_start(scr_d[w], hobuf[:])
                        dst4 = out_d[:, w0 : w0 + wout, :].rearrange(
                            "b tl (hb p) -> hb b tl p", hb=HB
                        )
                        scr4 = scr_d[w].rearrange("(hb b) tl p -> hb b tl p", hb=HB)
                        for hb in range(HB):
                            nc.sync.dma_start(dst4[hb], scr4[hb])
                for t in range(n_steps):
                    c, tl = t // ch, t % ch
                    if tl == 0 and next_chunk < nchunk:
                        gemm_work.append(emit_chunk(next_chunk))
                        next_chunk += 1
                        if not fused:
                            pump_gemm(10**9)

                    u_tile = u_tiles[c]
                    u_ap = u_tile[:, :, tl, :]  # [128, HB, BL] strided

                    dh = dhp.tile([128, 128], FP32, tag="dh")
                    nc.vector.tensor_mul(dh[:], h_prev[:], dbc_sb[:])

                    # ya[(hbr,r),(hbk,b)] = R-block-hbr . h-block-hbk: ONE
                    # indep matmul (no PSUM-accum chain -- chained accumulating
                    # matmuls cost ~5x in this env)
                    y_ps_full = yps.tile([128, 4, HB, BL], FP32, tag="yps")
                    ya = y_ps_full[:32, 0, :, :]
                    y_rep = y_ps_full[:32, 2, 0, :]
                    nc.tensor.matmul(ya, W1a_sb[:], h_prev[:], start=True, stop=True)
                    # mask off-diagonal block pairs, then col-tree-sum the HB
                    # groups: s8[(hbr,r),b] = partial_hbr[r,b]  (4 DVE ops;
                    # contiguous-stride adds beat a strided tensor_reduce here)
                    yt = bdp.tile([32, HB, BL], FP32, tag="m1")
                    tr = bdp.tile([32, 7, BL], FP32, tag="tr")
                    nc.vector.tensor_mul(yt[:], ya, mask_sb[:])
                    nc.vector.tensor_add(tr[:, 0:4, :], yt[:, 0:4, :], yt[:, 4:8, :])
                    nc.vector.tensor_add(tr[:, 4:6, :], tr[:, 0:2, :], tr[:, 2:4, :])
                    nc.vector.tensor_add(tr[:, 6:7, :], tr[:, 4:5, :], tr[:, 5:6, :])
                    # y_rep[(hbr',r),b] = sum_hbr s8[(hbr,r),b] = y[r,b] (tiny mm)
                    nc.tensor.matmul(
                        y_rep, P32_sb[:], tr[:, 6, :], start=True, stop=True
                    )

                    # bd = broadcast(y_rep) * mask  (block-diagonal [32, HB, BL])
                    bd = bdp.tile([32, HB, BL], BF16, tag="bd")
                    yap = y_rep
                    y_b = bass.AP(
                        tensor=yap.tensor,
                        offset=yap.offset,
                        ap=[yap.ap[0], [0, HB], yap.ap[1]],
                    )
                    bd_i = nc.vector.tensor_mul(bd[:], y_b, mask_sb[:])
                    s1 = dhp.tile([128, 128], FP32, tag="s1")
                    s1_i = nc.vector.tensor_add(s1[:], dh[:], u_ap)
                    # keep bd ahead of s1 on DVE: bd feeds the lr matmul on
                    # the critical path; s1 has slack until s2
                    tile.add_dep_helper(bd_i.ins, s1_i.ins, sync=False, reason="bd first")

                    lr = lps.tile([128, 128], FP32, tag="lps")
                    nc.tensor.matmul(lr[:], W2_sb[:], bd[:], start=True, stop=True)

                    # output path for the PREVIOUS step (keeps the PE transpose
                    # off this step's tanh -> y-matmul critical path)
                    if t > 0 and not no_out:
                        emit_out(t - 1)

                    # s = (dh + u) + lr on DVE (s1 runs parallel to the matmuls)
                    s_t = sp.tile([128, 128], FP32, tag="s")
                    nc.vector.tensor_add(s_t[:], s1[:], lr[:])

                    h_new = hp.tile([128, 128], FP32, tag="h")
                    nc.scalar.activation(h_new[:], s_t[:], TANH, bias=zb[:])

                    if not no_out:
                        h_hist[t] = h_new
                    h_prev = h_new
                    if fused:
                        pump_gemm(6)
                if not no_out:
                    emit_out(n_steps - 1)
                pump_gemm(10**9)
            if timing_reps:
                nc.sync.dma_start(tok_d[:], dbc_sb[:1, :4])
            loop_cm.__exit__(None, None, None)

    if strip:
        _strip_self_waits(nc, dma=(strip is True))
    nc.compile()
    return nc


_ENG_SEM = {
    "EngineType.PE": "PE_",
    "EngineType.DVE": "DVE_",
    "EngineType.Activation": "Activation_",
}


def _strip_self_waits(nc, dma=True):
    """trn2 compute instructions carry at most ONE sync wait.  Engines
    execute and retire their queues strictly in order, so a wait on the
    instruction's own engine semaphore (emitted by Tile for cross-step
    tile reuse) is redundant -- drop those when over the limit.  The
    dma=True mode additionally drops DMA WAW waits (NOT safe for the
    real program's out-DMA windows -- timing-only)."""
    import concourse.mybir as _mb

    over = []
    for b in nc.m.functions[0].blocks:
        for inst in b.instructions:
            si = inst.sync_info
            if not si or not si.on_wait or len(si.on_wait) <= 1:
                continue
            ty = type(inst).__name__
            keep = si.on_wait
            pfx = _ENG_SEM.get(str(getattr(inst, "engine", None)))
            if pfx is not None:
                keep = [w for w in keep if not w.ant_name.startswith(pfx)]
            if dma and ty == "InstDMACopy" and len(keep) > 1:
                # DMA WAW waits on other DMA-queue sems: every recycled DMA
                # target in this kernel is transitively ordered through the
                # kept engine-sem wait (xt: PE readers; hobuf: ACT copies),
                # and DRAM-out windows are disjoint regions.
                eng_waits = [w for w in keep if not w.ant_name.startswith("DMA")]
                if eng_waits:
                    keep = eng_waits
                else:
                    keep = keep[-1:]
            if len(keep) < len(si.on_wait):
                inst.sync_info = _mb.SyncInfo(on_wait=keep, on_update=si.on_update)
            if len(keep) > 1:
                over.append((inst.name, ty, [w.ant_name for w in keep]))
    if over:
        print(f"WARNING: {len(over)} instructions still have >1 wait:")
        for o in over[:10]:
            print("   ", o)


_PROG_CACHE = {}


def build_program_timed(n_steps=T, reps=8, **kw):
    return build_program(n_steps, timing_reps=reps, **kw)


STRIP = False  # strip tested slower (and dma-strip incorrect); keep full syncs


def _get_prog(n_steps=T, fused=True):
    key = (n_steps, fused, STRIP)
    if key not in _PROG_CACHE:
        _PROG_CACHE[key] = build_program(n_steps, fused, strip=STRIP)
    return _PROG_CACHE[key]


def make_core_inputs(x, h0, d, L, R, B, n_steps=T):
    """Host-side preprocessing -> list of per-core input dicts."""
    x = np.asarray(x, np.float32)
    h0 = np.asarray(h0, np.float32)
    d = np.asarray(d, np.float32)
    L = np.asarray(L, np.float32)
    R = np.asarray(R, np.float32)
    B = np.asarray(B, np.float32)

    import ml_dtypes

    bf16 = ml_dtypes.bfloat16
    # constants (replicated across cores)
    Bw = np.ascontiguousarray(B.reshape(HB, 128, H).transpose(1, 0, 2)).astype(bf16)
    # W1a[p, hbr*RANK+r] = R[r, hbr*128+p]  (lhsT for the one-shot ya matmul)
    Rr = R.reshape(RANK, HB, 128)  # [r, hbr, p]
    W1a = np.ascontiguousarray(Rr.transpose(2, 1, 0).reshape(128, 32))
    # P32[(hbr,r),(hbr',r')] = [r==r']: partition-sum of the 8 rank-groups
    P32 = np.tile(np.eye(RANK, dtype=np.float32), (HB, HB))
    # W2[hb2*RANK+r, p] = L[hb2*128+p, r]
    W2 = np.ascontiguousarray(
        L.reshape(HB, 128, RANK).transpose(0, 2, 1).reshape(32, 128)
    ).astype(bf16)
    mask = np.zeros((32, HB, BL), np.float32)
    for hb in range(HB):
        mask[hb * RANK : (hb + 1) * RANK, hb, :] = 1.0
    # dbc[p, hb*BL+b] = d[hb*128+p]
    dbc = np.ascontiguousarray(
        np.repeat(d.reshape(HB, 128).T[:, :, None], BL, axis=2).reshape(128, 128)
    )
    ident = np.eye(128, dtype=np.float32)

    in_maps = []
    for core in range(NCORES):
        sl = slice(core * BL, (core + 1) * BL)
        xs = x[sl, :n_steps]  # [BL, T, H]
        # xT[hbk, k, t*BL+b] = x[b, t, hbk*128+k]
        xT = np.ascontiguousarray(
            xs.reshape(BL, n_steps, HB, 128)
            .transpose(2, 3, 1, 0)
            .reshape(HB, 128, n_steps * BL)
        ).astype(bf16)
        h0s = h0[sl]  # [BL, H]
        h0A = np.ascontiguousarray(
            h0s.reshape(BL, HB, 128).transpose(2, 1, 0).reshape(128, 128)
        )
        in_maps.append(
            {
                "xT": xT,
                "Bw": Bw,
                "W1a": W1a,
                "P32": P32,
                "W2": W2,
                "mask": mask,
                "dbc": dbc,
                "ident": ident,
                "h0A": h0A,
            }
        )
    return in_maps


def gather_output(results, n_steps=T):
    """results: list of per-core dicts with 'out' [BL, T, H] -> [BATCH, T, H]."""
    return np.concatenate([np.asarray(r["out"]) for r in results], axis=0)


def kernel(x, h0, d, L, R, B):
    nc = _get_prog(T)
    in_maps = make_core_inputs(x, h0, d, L, R, B, T)
    res = run_bass_kernel_spmd(nc, in_maps, list(range(NCORES)))
    return gather_output(res.results, T)


if __name__ == "__main__":
    nc = build_program()
    print("built ok:", sum(1 for _ in nc.m.functions[0].body))



# revision 28
# speedup vs baseline: 1.5328x; 1.2073x over previous
"""DPLR-SSM block kernel for Trainium2 (8 NeuronCores, batch-data-parallel).

Computes, for the full inputs:
    xB = einsum("bth,hk->btk", x, B)
    h_{t+1} = tanh(d * h_t + (h_t @ R.T) @ L.T + xB[:, t])   (scan over t)
    out[:, t] = h_{t+1}

Sharding: batch 128 -> 16 per core (8 cores), params replicated.

Per-core device layout ("layout A"): state h lives in an SBUF tile
[128, 128] indexed [p, hb*16 + b] with h-index = hb*128 + p (b = local
batch, hb = h-block).  Per scan step:
  - y_rep [32,16] PSUM  = 8 PSUM-accumulated matmuls with column-replicated
    R weights  (y = R @ h, replicated over 8 partition groups)
  - bd [32,(8,16)] SBUF = broadcast(y_rep) * blockdiag_mask  (one DVE op)
  - lr [128,128] PSUM   = W2.T @ bd  (one matmul, constant [32,128] L weights)
                          += I.T @ u_t  (identity matmul, u from fused GEMM)
                          += I.T @ (d*h) (identity matmul; d*h on DVE)
  - h' = tanh(lr) on ScalarE, PSUM -> SBUF
  - PE-transpose h' -> PSUM -> copy -> SBUF -> DMA to out[b, t, :]
The xB GEMM is emitted interleaved with the scan so its matmuls fill the
PE idle slots of the latency-bound recurrence; u chunks (32 timesteps)
are double-buffered in SBUF and never round-trip through DRAM.

The GEMM runs in bf16 (x, B, and the W2/bd low-rank operands are rounded
host-side; PSUM accumulation stays fp32): trn2's PE streams fp32 moving
data at 1/4 rate (4 cycles/column), so bf16 cuts the dominant GEMM cost
4x. The recurrence state h and all elementwise math stay fp32; the
contractive recurrence keeps the bf16 input error saturated at ~1.3e-2
absmax-relative (verified at T=32/96/256 -- no growth with T).
"""

import sys

sys.path.insert(0, "/opt/trn_rl_repo")

import numpy as np

import concourse.bass as bass
import concourse.mybir as mybir
import concourse.tile as tile
from concourse import bacc
from concourse.bass_utils import run_bass_kernel_spmd

H = 1024
RANK = 4
BATCH = 128
T = 256
NCORES = 8
BL = BATCH // NCORES  # 16 local batches
HB = H // 128  # 8 h-blocks
CH = 32  # timesteps per GEMM chunk
WOUT = 8  # timesteps per output DMA window
FP32 = mybir.dt.float32
BF16 = mybir.dt.bfloat16


def build_program(
    n_steps=T,
    fused=True,
    strip=False,
    loops=1,
    timing_reps=0,
    no_gemm=False,
    no_out=False,
    y_one=False,
    gemm_split=1,
):
    """Build the single-core SPMD bass program."""
    ch = min(CH, n_steps)
    wout = min(WOUT, n_steps)
    assert n_steps % ch == 0
    assert n_steps % wout == 0
    nchunk = n_steps // ch
    nc = bacc.Bacc()

    # ---- DRAM I/O (per-core shard + host-preprocessed constants) ----
    if timing_reps:
        xT_d = nc.dram_tensor("xT", [HB, 128, ch * BL], BF16, kind="ExternalInput")
    else:
        xT_d = nc.dram_tensor(
            "xT", [HB, 128, n_steps * BL], BF16, kind="ExternalInput"
        )
    Bw_d = nc.dram_tensor("Bw", [128, HB, H], BF16, kind="ExternalInput")
    W1a_d = nc.dram_tensor("W1a", [128, 32], FP32, kind="ExternalInput")
    P32_d = nc.dram_tensor("P32", [32, 32], FP32, kind="ExternalInput")
    W2_d = nc.dram_tensor("W2", [32, 128], BF16, kind="ExternalInput")
    mask_d = nc.dram_tensor("mask", [32, HB, BL], FP32, kind="ExternalInput")
    dbc_d = nc.dram_tensor("dbc", [128, 128], FP32, kind="ExternalInput")
    id_d = nc.dram_tensor("ident", [128, 128], FP32, kind="ExternalInput")
    h0_d = nc.dram_tensor("h0A", [128, 128], FP32, kind="ExternalInput")
    okind = "Internal" if timing_reps else "ExternalOutput"
    out_d = nc.dram_tensor("out", [BL, n_steps, H], FP32, kind=okind)
    if timing_reps:
        tok_d = nc.dram_tensor("token", [1, 4], FP32, kind="ExternalOutput")
    scr_d = nc.dram_tensor("oscr", [n_steps // wout, 128, wout, 128], FP32)

    import contextlib

    with tile.TileContext(nc) as tc:
        loop_cm = (
            tc.For_i(
                0,
                loops,
                1,
                hint_engines=(
                    mybir.EngineType.PE,
                    mybir.EngineType.DVE,
                    mybir.EngineType.Activation,
                    mybir.EngineType.Pool,
                    mybir.EngineType.SP,
                ),
            )
            if loops > 1
            else contextlib.nullcontext()
        )
        with (
            tc.tile_pool(name="consts", bufs=1) as consts,
            tc.tile_pool(name="xt", bufs=24) as xtp,
            tc.tile_pool(name="uc", bufs=3) as ucp,
            tc.tile_pool(name="h", bufs=8) as hp,
            tc.tile_pool(name="dh", bufs=2) as dhp,
            tc.tile_pool(name="bd", bufs=2) as bdp,
            tc.tile_pool(name="sps", bufs=2, space="PSUM") as sp,
            tc.tile_pool(name="ho", bufs=2) as hop,
            tc.tile_pool(name="gps", bufs=2, space="PSUM") as gps,
            tc.tile_pool(name="yps", bufs=1, space="PSUM") as yps,
            tc.tile_pool(name="lps", bufs=1, space="PSUM") as lps,
            tc.tile_pool(name="tps", bufs=1, space="PSUM") as tps,
            tc.tile_pool(name="dps", bufs=1, space="PSUM") as dps,
        ):
            # ---- load constants ----
            B_sb = consts.tile([128, HB, H], BF16)
            nc.sync.dma_start(B_sb[:], Bw_d[:])
            W1a_sb = consts.tile([128, 32], FP32)
            nc.sync.dma_start(W1a_sb[:], W1a_d[:])
            P32_sb = consts.tile([32, 32], FP32)
            nc.sync.dma_start(P32_sb[:], P32_d[:])
            W2_sb = consts.tile([32, 128], BF16)
            nc.sync.dma_start(W2_sb[:], W2_d[:])
            mask_sb = consts.tile([32, HB, BL], FP32)
            nc.sync.dma_start(mask_sb[:], mask_d[:])
            dbc_sb = consts.tile([128, 128], FP32)
            nc.sync.dma_start(dbc_sb[:], dbc_d[:])
            I_sb = consts.tile([128, 128], FP32)
            nc.sync.dma_start(I_sb[:], id_d[:])
            h_prev = hp.tile([128, 128], FP32, tag="h")
            nc.sync.dma_start(h_prev[:], h0_d[:])

            # PE "wait absorber" touches: matmul can carry only ONE sync
            # wait on trn2, so teach PE's vector clock about every const
            # DMA queue up front (one dummy matmul per const).
            dummy_ps = dps.tile([128, 128], FP32, tag="dps")
            for cap in (B_sb[:, 0, 0:128], W1a_sb[:], P32_sb[:], W2_sb[:],
                        mask_sb[:], dbc_sb[:], I_sb[:], h_prev[:]):
                fs = cap.free_size()
                nc.tensor.matmul(dummy_ps[:fs, :fs], cap, cap, start=True, stop=True)
            # DVE/ACT const-touch absorbers (each carries one DMA-queue wait)
            sc1 = consts.tile([128, 1], FP32)
            sc2 = consts.tile([128, 1], FP32)
            sc3 = consts.tile([128, 1], FP32)
            nc.vector.tensor_copy(sc1[:1, :], dbc_sb[:1, :1])
            nc.vector.tensor_copy(sc2[:1, :], mask_sb[:1, :1, 0])
            nc.scalar.copy(sc3[:1, :], dbc_sb[:1, :1])
            zb = consts.tile([128, 1], FP32)
            nc.vector.memset(zb[:], 0.0)
            sc4 = consts.tile([128, 1], FP32)
            nc.scalar.copy(sc4[:1, :], h_prev[:1, :1])

            # ---- GEMM chunk emission (generator yields after each item) ----
            loop_ctx = loop_cm.__enter__()
            for _rep in range(max(1, timing_reps)):
                pdum = dps.tile([128, 128], FP32, tag="dps")
                u_tiles = [None] * nchunk

                def emit_chunk(c, prologue=False):
                    xts = []
                    for hbk in range(HB):
                        xt = xtp.tile([128, ch * BL], BF16, tag="xt")
                        xsl = (
                            xT_d[hbk, :, :]
                            if timing_reps
                            else xT_d[hbk, :, c * ch * BL : (c + 1) * ch * BL]
                        )
                        nc.sync.dma_start(xt[:], xsl)
                        xts.append(xt)
                        yield
                    u_tile = ucp.tile([128, HB, ch, BL], FP32, tag="uc")
                    u_tiles[c] = u_tile
                    hc = ch // gemm_split
                    for hbp in range(HB):
                        ps = gps.tile([128, ch, BL], FP32, tag="gps")
                        for hbk in range(HB):
                            for hf in range(gemm_split):
                                nc.tensor.matmul(
                                    ps[:, hf * hc : (hf + 1) * hc, :],
                                    B_sb[:, hbk, hbp * 128 : (hbp + 1) * 128],
                                    xts[hbk][:, hf * hc * BL : (hf + 1) * hc * BL],
                                    start=(hbk == 0 and hf == 0),
                                    stop=(hbk == HB - 1 and hf == gemm_split - 1),
                                )
                                yield
                        # copy psum -> u_tile[:, hbp] in halves on DVE (cheap
                        # engine in this env; avoids ACT<->DVE flip-flops)
                        q = ch // 2
                        for j in range(2):
                            dst = u_tile[:, hbp, j * q : (j + 1) * q, :]
                            piece = ps[:, j * q : (j + 1) * q, :]
                            nc.vector.tensor_copy(dst, piece)
                        if prologue:
                            # PE absorber: observe the copy's ACT tick so later
                            # GEMM matmuls' bank-WAR waits are pre-satisfied
                            nc.tensor.matmul(
                                dummy_ps[:32, :BL],
                                W1a_sb[:],
                                u_tile[:, hbp, 0, :],
                                start=True,
                                stop=True,
                            )
                        yield

                gemm_work = []  # list of generators, consumed round-robin

                def pump_gemm(n):
                    done = 0
                    while done < n and gemm_work:
                        try:
                            next(gemm_work[0])
                            done += 1
                        except StopIteration:
                            gemm_work.pop(0)

                # prologue: only chunk 0 before the scan; chunk 1 streams
                # during steps 0..ch-1 (needed first at step ch)
                gemm_work.append(emit_chunk(0, prologue=True))
                pump_gemm(10**9)
                if no_gemm:
                    for cc_ in range(1, nchunk):
                        u_tiles[cc_] = u_tiles[0]
                    next_chunk = nchunk
                elif nchunk > 1:
                    gemm_work.append(emit_chunk(1, prologue=True))
                    next_chunk = 2
                else:
                    next_chunk = 2

                # ---- the scan ----
                TANH = mybir.ActivationFunctionType.Tanh
                hobufs = [None, None]
                h_hist = {}

                def emit_out(t):
                    wl = t % wout
                    if wl == 0:
                        hobuf = hop.tile([128, wout, 128], FP32, tag="ho")
                        hobufs[(t // wout) % 2] = hobuf
                        # absorber: carry the stage-DMA's queue wait on a
                        # lone ACT op (every instr gets at most one sync wait)
                        nc.scalar.copy(hobuf[:1, 0, :1], dbc_sb[:1, :1])
                    hobuf = hobufs[(t // wout) % 2]
                    ht = tps.tile([128, 128], FP32, tag="tps")
                    nc.tensor.transpose(ht[:], h_hist.pop(t)[:], I_sb[:])
                    nc.scalar.copy(hobuf[:, wl, :], ht[:])
                    if wl == wout - 1:
                        w0, w = t - wl, t // wout
                        nc.sync.dma_start(scr_d[w], hobuf[:])
                        dst4 = out_d[:, w0 : w0 + wout, :].rearrange(
                            "b tl (hb p) -> hb b tl p", hb=HB
                        )
                        scr4 = scr_d[w].rearrange("(hb b) tl p -> hb b tl p", hb=HB)
                        for hb in range(HB):
                            nc.sync.dma_start(dst4[hb], scr4[hb])
                for t in range(n_steps):
                    c, tl = t // ch, t % ch
                    if tl == 0 and next_chunk < nchunk:
                        gemm_work.append(emit_chunk(next_chunk))
                        next_chunk += 1
                        if not fused:
                            pump_gemm(10**9)

                    u_tile = u_tiles[c]
                    u_ap = u_tile[:, :, tl, :]  # [128, HB, BL] strided

                    dh = dhp.tile([128, 128], FP32, tag="dh")
                    nc.vector.tensor_mul(dh[:], h_prev[:], dbc_sb[:])

                    # ya[(hbr,r),(hbk,b)] = R-block-hbr . h-block-hbk: ONE
                    # indep matmul (no PSUM-accum chain -- chained accumulating
                    # matmuls cost ~5x in this env)
                    y_ps_full = yps.tile([128, 4, HB, BL], FP32, tag="yps")
                    ya = y_ps_full[:32, 0, :, :]
                    y_rep = y_ps_full[:32, 2, 0, :]
                    nc.tensor.matmul(ya, W1a_sb[:], h_prev[:], start=True, stop=True)
                    # mask off-diagonal block pairs, then col-tree-sum the HB
                    # groups: s8[(hbr,r),b] = partial_hbr[r,b]  (4 DVE ops;
                    # contiguous-stride adds beat a strided tensor_reduce here)
                    yt = bdp.tile([32, HB, BL], FP32, tag="m1")
                    tr = bdp.tile([32, 7, BL], FP32, tag="tr")
                    nc.vector.tensor_mul(yt[:], ya, mask_sb[:])
                    nc.vector.tensor_add(tr[:, 0:4, :], yt[:, 0:4, :], yt[:, 4:8, :])
                    nc.vector.tensor_add(tr[:, 4:6, :], tr[:, 0:2, :], tr[:, 2:4, :])
                    nc.vector.tensor_add(tr[:, 6:7, :], tr[:, 4:5, :], tr[:, 5:6, :])
                    # y_rep[(hbr',r),b] = sum_hbr s8[(hbr,r),b] = y[r,b] (tiny mm)
                    nc.tensor.matmul(
                        y_rep, P32_sb[:], tr[:, 6, :], start=True, stop=True
                    )

                    # bd = broadcast(y_rep) * mask  (block-diagonal [32, HB, BL])
                    bd = bdp.tile([32, HB, BL], BF16, tag="bd")
                    yap = y_rep
                    y_b = bass.AP(
                        tensor=yap.tensor,
                        offset=yap.offset,
                        ap=[yap.ap[0], [0, HB], yap.ap[1]],
                    )
                    bd_i = nc.vector.tensor_mul(bd[:], y_b, mask_sb[:])
                    s1 = dhp.tile([128, 128], FP32, tag="s1")
                    s1_i = nc.vector.tensor_add(s1[:], dh[:], u_ap)
                    # keep bd ahead of s1 on DVE: bd feeds the lr matmul on
                    # the critical path; s1 has slack until s2
                    tile.add_dep_helper(bd_i.ins, s1_i.ins, sync=False, reason="bd first")

                    lr = lps.tile([128, 128], FP32, tag="lps")
                    nc.tensor.matmul(lr[:], W2_sb[:], bd[:], start=True, stop=True)

                    # output path for the PREVIOUS step (keeps the PE transpose
                    # off this step's tanh -> y-matmul critical path)
                    if t > 0 and not no_out:
                        emit_out(t - 1)

                    # s = (dh + u) + lr on DVE (s1 runs parallel to the matmuls)
                    s_t = sp.tile([128, 128], FP32, tag="s")
                    nc.vector.tensor_add(s_t[:], s1[:], lr[:])

                    h_new = hp.tile([128, 128], FP32, tag="h")
                    nc.scalar.activation(h_new[:], s_t[:], TANH, bias=zb[:])

                    if not no_out:
                        h_hist[t] = h_new
                    h_prev = h_new
                    if fused:
                        pump_gemm(6)
                if not no_out:
                    emit_out(n_steps - 1)
                pump_gemm(10**9)
            if timing_reps:
                nc.sync.dma_start(tok_d[:], dbc_sb[:1, :4])
            loop_cm.__exit__(None, None, None)

    if strip:
        _strip_self_waits(nc, dma=(strip is True))
    nc.compile()
    return nc


_ENG_SEM = {
    "EngineType.PE": "PE_",
    "EngineType.DVE": "DVE_",
    "EngineType.Activation": "Activation_",
}


def _strip_self_waits(nc, dma=True):
    """trn2 compute instructions carry at most ONE sync wait.  Engines
    execute and retire their queues strictly in order, so a wait on the
    instruction's own engine semaphore (emitted by Tile for cross-step
    tile reuse) is redundant -- drop those when over the limit.  The
    dma=True mode additionally drops DMA WAW waits (NOT safe for the
    real program's out-DMA windows -- timing-only)."""
    import concourse.mybir as _mb

    over = []
    for b in nc.m.functions[0].blocks:
        for inst in b.instructions:
            si = inst.sync_info
            if not si or not si.on_wait or len(si.on_wait) <= 1:
                continue
            ty = type(inst).__name__
            keep = si.on_wait
            pfx = _ENG_SEM.get(str(getattr(inst, "engine", None)))
            if pfx is not None:
                keep = [w for w in keep if not w.ant_name.startswith(pfx)]
            if dma and ty == "InstDMACopy" and len(keep) > 1:
                # DMA WAW waits on other DMA-queue sems: every recycled DMA
                # target in this kernel is transitively ordered through the
                # kept engine-sem wait (xt: PE readers; hobuf: ACT copies),
                # and DRAM-out windows are disjoint regions.
                eng_waits = [w for w in keep if not w.ant_name.startswith("DMA")]
                if eng_waits:
                    keep = eng_waits
                else:
                    keep = keep[-1:]
            if len(keep) < len(si.on_wait):
                inst.sync_info = _mb.SyncInfo(on_wait=keep, on_update=si.on_update)
            if len(keep) > 1:
                over.append((inst.name, ty, [w.ant_name for w in keep]))
    if over:
        print(f"WARNING: {len(over)} instructions still have >1 wait:")
        for o in over[:10]:
            print("   ", o)


_PROG_CACHE = {}


def build_program_timed(n_steps=T, reps=8, **kw):
    return build_program(n_steps, timing_reps=reps, **kw)


STRIP = False  # strip tested slower (and dma-strip incorrect); keep full syncs


def _get_prog(n_steps=T, fused=True):
    key = (n_steps, fused, STRIP)
    if key not in _PROG_CACHE:
        _PROG_CACHE[key] = build_program(n_steps, fused, strip=STRIP)
    return _PROG_CACHE[key]


def make_core_inputs(x, h0, d, L, R, B, n_steps=T):
    """Host-side preprocessing -> list of per-core input dicts."""
    x = np.asarray(x, np.float32)
    h0 = np.asarray(h0, np.float32)
    d = np.asarray(d, np.float32)
    L = np.asarray(L, np.float32)
    R = np.asarray(R, np.float32)
    B = np.asarray(B, np.float32)

    import ml_dtypes

    bf16 = ml_dtypes.bfloat16
    # constants (replicated across cores)
    Bw = np.ascontiguousarray(B.reshape(HB, 128, H).transpose(1, 0, 2)).astype(bf16)
    # W1a[p, hbr*RANK+r] = R[r, hbr*128+p]  (lhsT for the one-shot ya matmul)
    Rr = R.reshape(RANK, HB, 128)  # [r, hbr, p]
    W1a = np.ascontiguousarray(Rr.transpose(2, 1, 0).reshape(128, 32))
    # P32[(hbr,r),(hbr',r')] = [r==r']: partition-sum of the 8 rank-groups
    P32 = np.tile(np.eye(RANK, dtype=np.float32), (HB, HB))
    # W2[hb2*RANK+r, p] = L[hb2*128+p, r]
    W2 = np.ascontiguousarray(
        L.reshape(HB, 128, RANK).transpose(0, 2, 1).reshape(32, 128)
    ).astype(bf16)
    mask = np.zeros((32, HB, BL), np.float32)
    for hb in range(HB):
        mask[hb * RANK : (hb + 1) * RANK, hb, :] = 1.0
    # dbc[p, hb*BL+b] = d[hb*128+p]
    dbc = np.ascontiguousarray(
        np.repeat(d.reshape(HB, 128).T[:, :, None], BL, axis=2).reshape(128, 128)
    )
    ident = np.eye(128, dtype=np.float32)

    in_maps = []
    for core in range(NCORES):
        sl = slice(core * BL, (core + 1) * BL)
        xs = x[sl, :n_steps]  # [BL, T, H]
        # xT[hbk, k, t*BL+b] = x[b, t, hbk*128+k]
        xT = np.ascontiguousarray(
            xs.reshape(BL, n_steps, HB, 128)
            .transpose(2, 3, 1, 0)
            .reshape(HB, 128, n_steps * BL)
        ).astype(bf16)
        h0s = h0[sl]  # [BL, H]
        h0A = np.ascontiguousarray(
            h0s.reshape(BL, HB, 128).transpose(2, 1, 0).reshape(128, 128)
        )
        in_maps.append(
            {
                "xT": xT,
                "Bw": Bw,
                "W1a": W1a,
                "P32": P32,
                "W2": W2,
                "mask": mask,
                "dbc": dbc,
                "ident": ident,
                "h0A": h0A,
            }
        )
    return in_maps


def gather_output(results, n_steps=T):
    """results: list of per-core dicts with 'out' [BL, T, H] -> [BATCH, T, H]."""
    return np.concatenate([np.asarray(r["out"]) for r in results], axis=0)


def kernel(x, h0, d, L, R, B):
    nc = _get_prog(T)
    in_maps = make_core_inputs(x, h0, d, L, R, B, T)
    res = run_bass_kernel_spmd(nc, in_maps, list(range(NCORES)))
    return gather_output(res.results, T)


if __name__ == "__main__":
    nc = build_program()
    print("built ok:", sum(1 for _ in nc.m.functions[0].body))

